# revision 9
# baseline (speedup 1.0000x reference)
"""Trainium2 Bass kernel for nn_Light_Spattention (linearized attention / GNN
message passing).

Math (per (b,t) slice, x: [N, F], N=2048 nodes, F=256 features, 4 heads x 64):
    q = x @ Q ; k = x @ K
    summary_h = k_h^T @ x_h            (contract nodes)
    attn_h    = q_h @ summary_h / N
    out       = sig(alpha_h) * x + sig(beta_h) * attn_h

Refactored via the Gram matrix:
    G    = x^T x                       [256, 256]
    P    = K^T G ; Sig_h = P[h-block diag]
    Wa   = Qs @ Sig_bd                 (Qs = Q col-scaled by sig(beta)/N)
    attn = x @ Wa                      -> written transposed: attn^T = Wa^T x^T
    out  = sig(alpha) * x + attn       (computed on the host from exact x)

Per-core work = 6 of the 48 (b,t) slices (pure data parallel, no collectives).

The two big matmuls (Gram, attn) run as fp8e4m3 DoubleRow matmuls: each
instruction contracts TWO 128-row K-slabs at 0.5 cycles/row -- 4x the
bf16/fp32r rate.  fp8 quantization error is cancelled with residual passes
(x = x8 + r8, both e4m3; Wa = wa8 + war8) accumulated into the same PSUM
group, giving ~bf16 accuracy at fp8 speed:
    G    = x8^T x8 [+ x8^T r8 + r8^T x8]           (GRAM_CORR)
    attn = (wa8 + war8)^T x8t + wa8^T r8t          (3 passes)
The kernel is DMA-bandwidth-bound (shared 360 GB/s), so inputs arrive as
packed fp8 (x8 node-major for Gram; x8^T/r8^T feature-major pairs for attn)
and the output leaves as bf16 attn^T; the host adds sig(alpha)*x in fp32 and
transposes back.  P/Wa small matmuls run in bf16.  PSUM evictions are split
across DVE, ACT and GPSIMD.  PE work of slice i+1 (Gram/P/W) is woven between
slice i's attn chunk groups to keep the PE queue dense.
"""

import ml_dtypes
import numpy as np

import concourse.bass as bass  # noqa: F401
import concourse.tile as tile
from concourse import bacc, mybir
from concourse.bass_utils import run_bass_kernel_spmd

B, T, NN, DIM, HEAD = 4, 12, 2048, 256, 4
HD = DIM // HEAD            # 64
BT = B * T                  # 48
N_CORES = 8
SL = BT // N_CORES          # 6 slices per core
NT = NN // 128              # 16 node tiles per slice
EC = DIM // 128             # 2 feature chunks of 128
NCH = NN // 256             # 8 node column chunks for attn^T

F32 = mybir.dt.float32
F8 = mybir.dt.float8e4
BF16 = mybir.dt.bfloat16
DR = mybir.MatmulPerfMode.DoubleRow

# Gram residual correction: True = 3 DoubleRow passes (x8^T x8 + x8^T r8 +
# r8^T x8, needs the r8 upload), False = single raw x8^T x8 pass.
# Measured end-to-end rel err: True = 5.1e-3, False = 1.29e-2 (gate: 2e-2).
GRAM_CORR = False


def build_nc():
    nc = bacc.Bacc(None, target_bir_lowering=False)

    x8_d = nc.dram_tensor("x8", [SL, 128, NT * DIM], F8, kind="ExternalInput")
    if GRAM_CORR:
        r8_d = nc.dram_tensor("r8", [SL, 128, NT * DIM], F8, kind="ExternalInput")
    xtr_d = nc.dram_tensor("xtr", [SL, 128, 2, EC * NN], F8, kind="ExternalInput")
    kw_d = nc.dram_tensor("kw", [DIM, DIM], BF16, kind="ExternalInput")
    qst_d = nc.dram_tensor("qst", [DIM, DIM], BF16, kind="ExternalInput")
    zed_d = nc.dram_tensor("zed", [128, EC * DIM], BF16, kind="ExternalInput")
    out_d = nc.dram_tensor("out", [SL, 128, EC, NN], BF16, kind="ExternalOutput")

    with tile.TileContext(nc) as tc:
        with (
            tc.tile_pool(name="consts", bufs=1) as consts,
            tc.tile_pool(name="xin", bufs=3) as xin,
            tc.tile_pool(name="small", bufs=2) as small,
            tc.tile_pool(name="outp", bufs=2) as outp,
            tc.tile_pool(name="ps_g", bufs=2, space="PSUM") as ps_g,
            tc.tile_pool(name="ps_s", bufs=1, space="PSUM") as ps_s,
            tc.tile_pool(name="ps_a", bufs=4, space="PSUM") as ps_a,
        ):
            st = {}  # per-slice emission state

            def load_consts():
                kw = consts.tile([128, EC, DIM], BF16)
                nc.scalar.dma_start(
                    out=kw, in_=kw_d.rearrange("(c p) j -> p c j", p=128)
                )
                qst = consts.tile([128, EC, DIM], BF16)
                nc.scalar.dma_start(
                    out=qst, in_=qst_d.rearrange("(c p) e -> p c e", p=128)
                )
                # block-diagonal summary holders (double-buffered);
                # off-diagonal blocks stay zero forever
                sbd = []
                for s in range(2):
                    t = consts.tile([128, EC, DIM], BF16, name=f"sbd{s}")
                    nc.scalar.dma_start(
                        out=t, in_=zed_d.rearrange("p (c d) -> p c d", c=EC)
                    )
                    sbd.append(t)
                return kw, qst, sbd

            def dma_in(i):
                if i >= SL:
                    return
                s = st.setdefault(i, {})
                s["x8"] = xin.tile([128, NT, DIM], F8, tag="x8", name=f"x8_{i}")
                nc.sync.dma_start(
                    out=s["x8"], in_=x8_d[i].rearrange("p (t d) -> p t d", t=NT)
                )
                if GRAM_CORR:
                    s["r8"] = xin.tile([128, NT, DIM], F8, tag="r8", name=f"r8_{i}")
                    nc.sync.dma_start(
                        out=s["r8"], in_=r8_d[i].rearrange("p (t d) -> p t d", t=NT)
                    )
                s["xtr"] = xin.tile([128, 2, EC, NN], F8, tag="xtr", name=f"xtr{i}")
                nc.sync.dma_start(
                    out=s["xtr"], in_=xtr_d[i].rearrange("p w (c n) -> p w c n", c=EC)
                )

            def gram(i):
                """Gram accumulation: per feature chunk ecc one PSUM group of
                8 (or 24) DoubleRow matmuls over node-tile pairs."""
                if i >= SL:
                    return
                s = st[i]
                g_ps = ps_g.tile([128, EC, DIM], F32, tag="g", name=f"g{i}")
                s["g_ps"] = g_ps
                passes = [("x8", "x8")]
                if GRAM_CORR:
                    passes += [("x8", "r8"), ("r8", "x8")]
                nmm = len(passes) * NT // 2
                for ecc in range(EC):
                    k = 0
                    for lt, rt in passes:
                        for q in range(NT // 2):
                            nc.tensor.matmul(
                                g_ps[:, ecc, :],
                                s[lt][:, 2 * q : 2 * q + 2, ecc * 128 : (ecc + 1) * 128],
                                s[rt][:, 2 * q : 2 * q + 2, :],
                                start=(k == 0),
                                stop=(k == nmm - 1),
                                perf_mode=DR,
                            )
                            k += 1

            def g_evict(i):
                if i >= SL:
                    return
                s = st[i]
                g_sb = small.tile([128, EC, DIM], BF16, tag="g_sb", name=f"gs{i}")
                nc.vector.tensor_copy(
                    out=g_sb.rearrange("p c d -> p (c d)"),
                    in_=s["g_ps"].rearrange("p c d -> p (c d)"),
                )
                s["g_sb"] = g_sb

            def p_mm(i):
                if i >= SL:
                    return
                s = st[i]
                p_ps = ps_s.tile([128, EC, DIM], F32, tag="p", name=f"p{i}")
                s["p_ps"] = p_ps
                for jc in range(EC):
                    for ecc in range(EC):
                        nc.tensor.matmul(
                            p_ps[:, jc, :],
                            kw[:, ecc, jc * 128 : (jc + 1) * 128],
                            s["g_sb"][:, ecc, :],
                            start=(ecc == 0),
                            stop=(ecc == EC - 1),
                        )

            def sig_extract(i):
                if i >= SL:
                    return
                s = st[i]
                sb = sbd[i % 2]
                s["sbd"] = sb
                for h in range(HEAD):
                    jc, r = divmod(h, 2)
                    r0 = r * HD
                    src = s["p_ps"][r0 : r0 + HD, jc, h * HD : (h + 1) * HD]
                    dst = sb[r0 : r0 + HD, jc, h * HD : (h + 1) * HD]
                    if h % 2 == 0:
                        nc.vector.tensor_copy(out=dst, in_=src)
                    else:
                        nc.scalar.copy(out=dst, in_=src)

            def w_mm(i):
                if i >= SL:
                    return
                s = st[i]
                w_ps = ps_s.tile([128, EC, DIM], F32, tag="w", name=f"w{i}")
                s["w_ps"] = w_ps
                for ecc in range(EC):
                    for sc in range(EC):
                        nc.tensor.matmul(
                            w_ps[:, ecc, :],
                            qst[:, sc, ecc * 128 : (ecc + 1) * 128],
                            s["sbd"][:, sc, :],
                            start=(sc == 0),
                            stop=(sc == EC - 1),
                        )

            def w_evict(i):
                if i >= SL:
                    return
                s = st[i]
                wa8 = small.tile([128, EC, DIM], F8, tag="wa8", name=f"wa{i}")
                war8 = small.tile([128, EC, DIM], F8, tag="war8", name=f"wr{i}")
                wps_f = s["w_ps"].rearrange("p c d -> p (c d)")
                nc.scalar.copy(out=wa8.rearrange("p c d -> p (c d)"), in_=wps_f)
                nc.vector.tensor_sub(
                    war8.rearrange("p c d -> p (c d)"),
                    wps_f,
                    wa8.rearrange("p c d -> p (c d)"),
                )
                s["wa8"] = wa8
                s["war8"] = war8
                s["out_sb"] = outp.tile([128, EC, NN], BF16, tag="o", name=f"o{i}")

            def attn_bps(i, bps):
                """attn^T chunk-pair groups: bank pair bp covers output
                feature half eo = (2*bp)//NCH, node columns 512*bp'..+512.
                Each 256-col chunk is one 3-pass DoubleRow PSUM group; both
                chunks of a bank evict in one 512-wide op."""
                if i < 0:
                    return
                s = st[i]
                for bp in bps:
                    bank = ps_a.tile([128, 2, 256], F32, tag="a", name=f"a{i}_{bp}")
                    eo = (2 * bp) // NCH
                    for j in range(2):
                        c0 = ((2 * bp + j) % NCH) * 256
                        mms = [
                            (s["wa8"], 0),
                            (s["wa8"], 1),
                            (s["war8"], 0),
                        ]
                        for k, (lt, w) in enumerate(mms):
                            nc.tensor.matmul(
                                bank[:, j, :],
                                lt[:, :, eo * 128 : (eo + 1) * 128],
                                s["xtr"][:, w, :, c0 : c0 + 256],
                                start=(k == 0),
                                stop=(k == len(mms) - 1),
                                perf_mode=DR,
                            )
                    c0 = ((2 * bp) % NCH) * 256
                    dst = s["out_sb"][:, eo, c0 : c0 + 512]
                    src = bank.rearrange("p a b -> p (a b)")
                    if bp % 2 == 0:
                        nc.vector.tensor_copy(out=dst, in_=src)
                    else:
                        nc.scalar.copy(out=dst, in_=src)
                    if bp == 3 or bp == 7:
                        eo = bp // 4
                        nc.scalar.dma_start(
                            out=out_d[i, :, eo, :], in_=s["out_sb"][:, eo, :]
                        )

            # --- software pipeline ---
            dma_in(0)
            kw, qst, sbd = load_consts()
            dma_in(1)
            gram(0)
            g_evict(0)
            p_mm(0)
            sig_extract(0)
            w_mm(0)
            w_evict(0)
            for i in range(SL):
                dma_in(i + 2)
                attn_bps(i, [0, 1])
                gram(i + 1)
                attn_bps(i, [2, 3])
                g_evict(i + 1)
                p_mm(i + 1)
                attn_bps(i, [4, 5])
                sig_extract(i + 1)
                w_mm(i + 1)
                attn_bps(i, [6, 7])
                w_evict(i + 1)

    nc.finalize()
    return nc


F8NP = ml_dtypes.float8_e4m3


def _host_prep(x, Q, K, alpha, beta):
    x = np.asarray(x, dtype=np.float32).reshape(BT, NN, DIM)
    Q = np.asarray(Q, dtype=np.float32)
    K = np.asarray(K, dtype=np.float32)
    sb = (1.0 / (1.0 + np.exp(-np.asarray(beta, dtype=np.float32)))).reshape(HEAD)

    scale_cols = np.repeat(sb / NN, HD).astype(np.float32)        # [256]
    qs = Q * scale_cols[None, :]
    qst = np.ascontiguousarray(qs.T.astype(ml_dtypes.bfloat16))
    kw = np.ascontiguousarray(K.astype(ml_dtypes.bfloat16))
    zed = np.zeros((128, EC * DIM), dtype=ml_dtypes.bfloat16)

    in_maps = []
    for c in range(N_CORES):
        xs = x[c * SL : (c + 1) * SL]                              # [SL, NN, DIM]
        x8 = xs.astype(F8NP)
        r = xs - x8.astype(np.float32)
        r8 = r.astype(F8NP)

        def pack_nm(a):  # node-major [SL, NN, DIM] -> [SL, 128, NT*DIM]
            return np.ascontiguousarray(
                a.reshape(SL, NT, 128, DIM).transpose(0, 2, 1, 3).reshape(SL, 128, -1)
            )

        def pack_fm(a):  # feature-major [SL, NN, DIM] -> [SL, 128, EC*NN]
            at = a.transpose(0, 2, 1)                              # [SL, DIM, NN]
            return at.reshape(SL, EC, 128, NN).transpose(0, 2, 1, 3).reshape(
                SL, 128, -1
            )

        xtr = np.ascontiguousarray(
            np.stack([pack_fm(x8), pack_fm(r8)], axis=2)
        )                                                          # [SL, 128, 2, EC*NN]
        m = {"x8": pack_nm(x8), "xtr": xtr, "kw": kw, "qst": qst, "zed": zed}
        if GRAM_CORR:
            m["r8"] = pack_nm(r8)
        in_maps.append(m)
    return in_maps


def run(x, Q, K, alpha, beta, **spmd_kwargs):
    """Build, run on 8 cores, gather. Returns (out, BassKernelResults, nc)."""
    in_maps = _host_prep(x, Q, K, alpha, beta)
    nc = build_nc()
    res = run_bass_kernel_spmd(nc, in_maps, core_ids=list(range(N_CORES)), **spmd_kwargs)

    xf = np.asarray(x, dtype=np.float32)
    sa = (1.0 / (1.0 + np.exp(-np.asarray(alpha, dtype=np.float32)))).reshape(HEAD)
    sa_cols = np.repeat(sa, HD).astype(np.float32)                # [256]

    # device out: [SL, 128, EC, NN] bf16 = attn^T; host restores node-major
    attn = np.empty((BT, NN, DIM), dtype=np.float32)
    for c in range(N_CORES):
        a = res.results[c]["out"].astype(np.float32)              # [SL,128,EC,NN]
        attn[c * SL : (c + 1) * SL] = (
            a.transpose(0, 2, 1, 3).reshape(SL, DIM, NN).transpose(0, 2, 1)
        )
    out = sa_cols[None, None, None, :] * xf.reshape(B, T, NN, DIM) + attn.reshape(
        B, T, NN, DIM
    )
    return np.ascontiguousarray(out), res, nc


def kernel(x, Q, K, alpha, beta):
    out, _, _ = run(x, Q, K, alpha, beta)
    return out


# revision 11
# speedup vs baseline: 1.0164x; 1.0164x over previous
"""Trainium2 Bass kernel for nn_Light_Spattention (linearized attention / GNN
message passing).

Math (per (b,t) slice, x: [N, F], N=2048 nodes, F=256 features, 4 heads x 64):
    q = x @ Q ; k = x @ K
    summary_h = k_h^T @ x_h            (contract nodes)
    attn_h    = q_h @ summary_h / N
    out       = sig(alpha_h) * x + sig(beta_h) * attn_h

Refactored via the Gram matrix, with all weight-only algebra folded on the
host into per-head constants M_h = (Q_h * sig(beta_h)/N) @ K_h^T [256,256]:
    G          = x^T x                 [256, 256]
    Wa[:, h]   = M_h @ G[:, h-block]
    attn       = x @ Wa               -> written transposed: attn^T = Wa^T x^T
    out        = sig(alpha) * x + attn (computed on the host from exact x)

Per-core work = 6 of the 48 (b,t) slices (pure data parallel, no collectives).

The two big matmuls (Gram, attn) run as fp8e4m3 DoubleRow matmuls: each
instruction contracts TWO 128-row K-slabs at 0.5 cycles/row -- 4x the
bf16/fp32r rate.  fp8 quantization error on the attn side is cancelled with
residual passes (x = x8 + r8 both e4m3 feature-major; Wa = wa8 + war8 from
PSUM), giving near-bf16 accuracy at fp8 speed:
    attn = (wa8 + war8)^T x8t + wa8^T r8t          (3 passes, one PSUM group)
The kernel sits at the DMA/compute ridge, so inputs arrive as packed fp8
(x8 node-major for Gram; x8^T/r8^T feature-major pairs for attn) and the
output leaves as bf16 attn^T; the host adds sig(alpha)*x in fp32 and
transposes back.  The W matmul runs in bf16.  PSUM evictions alternate
between DVE and ACT; attn accumulates into 2-bank PSUM tiles so each
eviction is one 1024-wide op.  PE work of slice i+1 (Gram/W) is woven
between slice i's attn tiles to keep the PE queue dense.
"""

import ml_dtypes
import numpy as np

import concourse.bass as bass  # noqa: F401
import concourse.tile as tile
from concourse import bacc, mybir
from concourse.bass_utils import run_bass_kernel_spmd

B, T, NN, DIM, HEAD = 4, 12, 2048, 256, 4
HD = DIM // HEAD            # 64
BT = B * T                  # 48
N_CORES = 8
SL = BT // N_CORES          # 6 slices per core
NT = NN // 128              # 16 node tiles per slice
EC = DIM // 128             # 2 feature chunks of 128
NCH = NN // 256             # 8 node column chunks for attn^T

F32 = mybir.dt.float32
F8 = mybir.dt.float8e4
BF16 = mybir.dt.bfloat16
DR = mybir.MatmulPerfMode.DoubleRow

# Gram residual correction: True = 3 DoubleRow passes (x8^T x8 + x8^T r8 +
# r8^T x8, needs the r8 upload), False = single raw x8^T x8 pass.
# Measured end-to-end rel err: True = 4.6e-3, False = 1.27e-2 (gate: 2e-2).
GRAM_CORR = False


def build_nc():
    nc = bacc.Bacc(None, target_bir_lowering=False)

    x8_d = nc.dram_tensor("x8", [SL, 128, NT * DIM], F8, kind="ExternalInput")
    if GRAM_CORR:
        r8_d = nc.dram_tensor("r8", [SL, 128, NT * DIM], F8, kind="ExternalInput")
    xtr_d = nc.dram_tensor("xtr", [SL, 128, 2, EC * NN], F8, kind="ExternalInput")
    mh_d = nc.dram_tensor("mh", [128, HEAD * EC * DIM], BF16, kind="ExternalInput")
    out_d = nc.dram_tensor("out", [SL, 128, EC, NN], BF16, kind="ExternalOutput")

    with tile.TileContext(nc) as tc:
        with (
            tc.tile_pool(name="consts", bufs=1) as consts,
            tc.tile_pool(name="xin", bufs=3) as xin,
            tc.tile_pool(name="small", bufs=2) as small,
            tc.tile_pool(name="outp", bufs=2) as outp,
            tc.tile_pool(name="ps_g", bufs=1, space="PSUM") as ps_g,
            tc.tile_pool(name="ps_s", bufs=1, space="PSUM") as ps_s,
            tc.tile_pool(name="ps_a", bufs=3, space="PSUM") as ps_a,
        ):
            st = {}  # per-slice emission state
            mh = None

            def load_consts():
                t = consts.tile([128, HEAD, EC, DIM], BF16)
                nc.scalar.dma_start(
                    out=t, in_=mh_d.rearrange("p (h c m) -> p h c m", h=HEAD, c=EC)
                )
                return t

            def dma_in(i):
                if i >= SL:
                    return
                s = st.setdefault(i, {})
                s["x8"] = xin.tile([128, NT, DIM], F8, tag="x8", name=f"x8_{i}")
                nc.sync.dma_start(
                    out=s["x8"], in_=x8_d[i].rearrange("p (t d) -> p t d", t=NT)
                )
                if GRAM_CORR:
                    s["r8"] = xin.tile([128, NT, DIM], F8, tag="r8", name=f"r8_{i}")
                    nc.sync.dma_start(
                        out=s["r8"], in_=r8_d[i].rearrange("p (t d) -> p t d", t=NT)
                    )
                s["xtr"] = xin.tile([128, 2, EC, NN], F8, tag="xtr", name=f"xtr{i}")
                nc.sync.dma_start(
                    out=s["xtr"], in_=xtr_d[i].rearrange("p w (c n) -> p w c n", c=EC)
                )

            def gram(i):
                """Gram accumulation: per feature chunk ecc one PSUM group of
                8 (or 24) DoubleRow matmuls over node-tile pairs."""
                if i >= SL:
                    return
                s = st[i]
                g_ps = ps_g.tile([128, EC, DIM], F32, tag="g", name=f"g{i}")
                s["g_ps"] = g_ps
                passes = [("x8", "x8")]
                if GRAM_CORR:
                    passes += [("x8", "r8"), ("r8", "x8")]
                nmm = len(passes) * NT // 2
                for ecc in range(EC):
                    k = 0
                    for lt, rt in passes:
                        for q in range(NT // 2):
                            nc.tensor.matmul(
                                g_ps[:, ecc, :],
                                s[lt][:, 2 * q : 2 * q + 2, ecc * 128 : (ecc + 1) * 128],
                                s[rt][:, 2 * q : 2 * q + 2, :],
                                start=(k == 0),
                                stop=(k == nmm - 1),
                                perf_mode=DR,
                            )
                            k += 1

            def g_evict(i):
                """G psum -> bf16 sbuf in two column halves (DVE then ACT) so
                the W matmul's first head groups can start early."""
                if i >= SL:
                    return
                s = st[i]
                g_sb = small.tile([128, EC, DIM], BF16, tag="g_sb", name=f"gs{i}")
                nc.vector.tensor_copy(
                    out=g_sb[:, :, 0:128], in_=s["g_ps"][:, :, 0:128]
                )
                nc.scalar.copy(out=g_sb[:, :, 128:256], in_=s["g_ps"][:, :, 128:256])
                s["g_sb"] = g_sb

            def w_mm(i):
                """Wa[:, h-block] = M_h @ G[:, h-block]; 8 sequential PSUM
                groups (h, oc) of 2 kc-matmuls in one bank."""
                if i >= SL:
                    return
                s = st[i]
                w_ps = ps_s.tile([128, EC, DIM], F32, tag="w", name=f"w{i}")
                s["w_ps"] = w_ps
                for h in range(HEAD):
                    for oc in range(EC):
                        for kc in range(EC):
                            nc.tensor.matmul(
                                w_ps[:, oc, h * HD : (h + 1) * HD],
                                mh[:, h, kc, oc * 128 : (oc + 1) * 128],
                                s["g_sb"][:, kc, h * HD : (h + 1) * HD],
                                start=(kc == 0),
                                stop=(kc == EC - 1),
                            )

            def w_evict(i):
                if i >= SL:
                    return
                s = st[i]
                wa8 = small.tile([128, EC, DIM], F8, tag="wa8", name=f"wa{i}")
                war8 = small.tile([128, EC, DIM], F8, tag="war8", name=f"wr{i}")
                wps_f = s["w_ps"].rearrange("p c d -> p (c d)")
                nc.scalar.copy(out=wa8.rearrange("p c d -> p (c d)"), in_=wps_f)
                nc.vector.tensor_sub(
                    war8.rearrange("p c d -> p (c d)"),
                    wps_f,
                    wa8.rearrange("p c d -> p (c d)"),
                )
                s["wa8"] = wa8
                s["war8"] = war8
                s["out_sb"] = outp.tile([128, EC, NN], BF16, tag="o", name=f"o{i}")

            def attn_tiles(i, tiles):
                """attn^T: 2-bank PSUM tiles, 4 chunk groups each.  Chunk c
                (0..15): output feature half eo = c // NCH, node columns
                256*(c % NCH).  Each chunk is a 3-pass DoubleRow group; each
                tile evicts as one 1024-wide op, DVE/ACT alternating."""
                if i < 0:
                    return
                s = st[i]
                for t in tiles:
                    bank = ps_a.tile([128, 4, 256], F32, tag="a", name=f"a{i}_{t}")
                    eo = (4 * t) // NCH
                    for j in range(4):
                        c0 = ((4 * t + j) % NCH) * 256
                        mms = [
                            (s["wa8"], 0),
                            (s["wa8"], 1),
                            (s["war8"], 0),
                        ]
                        for k, (lt, w) in enumerate(mms):
                            nc.tensor.matmul(
                                bank[:, j, :],
                                lt[:, :, eo * 128 : (eo + 1) * 128],
                                s["xtr"][:, w, :, c0 : c0 + 256],
                                start=(k == 0),
                                stop=(k == len(mms) - 1),
                                perf_mode=DR,
                            )
                    c0 = ((4 * t) % NCH) * 1024
                    dst = s["out_sb"][:, eo, c0 : c0 + 1024]
                    src = bank.rearrange("p a b -> p (a b)")
                    if t % 2 == 0:
                        nc.vector.tensor_copy(out=dst, in_=src)
                    else:
                        nc.scalar.copy(out=dst, in_=src)
                    if i == SL - 1:
                        # tail: quarter-DMA per tile so the last transfers
                        # start as soon as each eviction lands
                        nc.sync.dma_start(
                            out=out_d[i, :, eo, c0 : c0 + 1024],
                            in_=s["out_sb"][:, eo, c0 : c0 + 1024],
                        )
                    elif t == 1 or t == 3:
                        eo = t // 2
                        nc.sync.dma_start(
                            out=out_d[i, :, eo, :], in_=s["out_sb"][:, eo, :]
                        )

            # --- software pipeline ---
            dma_in(0)
            mh = load_consts()
            dma_in(1)
            gram(0)
            g_evict(0)
            w_mm(0)
            w_evict(0)
            for i in range(SL):
                dma_in(i + 2)
                attn_tiles(i, [0, 1])
                gram(i + 1)
                g_evict(i + 1)
                attn_tiles(i, [2])
                w_mm(i + 1)
                attn_tiles(i, [3])
                w_evict(i + 1)

    nc.finalize()
    return nc


F8NP = ml_dtypes.float8_e4m3


def _host_prep(x, Q, K, alpha, beta):
    x = np.asarray(x, dtype=np.float32).reshape(BT, NN, DIM)
    Q = np.asarray(Q, dtype=np.float32)
    K = np.asarray(K, dtype=np.float32)
    sb = (1.0 / (1.0 + np.exp(-np.asarray(beta, dtype=np.float32)))).reshape(HEAD)

    # per-head folded weights M_h = (Q_h * sig(beta_h)/N) @ K_h^T, stored as
    # mh[p, (h, kc, m)] = M_h[m, p + 128*kc]  (lhsT layout for W = M_h @ G_h)
    mt = np.stack(
        [
            (
                (Q[:, h * HD : (h + 1) * HD].astype(np.float64) * (sb[h] / NN))
                @ K[:, h * HD : (h + 1) * HD].T.astype(np.float64)
            ).T
            for h in range(HEAD)
        ]
    ).astype(ml_dtypes.bfloat16)                               # [H, f, m]
    mh = np.ascontiguousarray(
        mt.reshape(HEAD, EC, 128, DIM).transpose(2, 0, 1, 3).reshape(128, -1)
    )

    in_maps = []
    for c in range(N_CORES):
        xs = x[c * SL : (c + 1) * SL]                          # [SL, NN, DIM]
        x8 = xs.astype(F8NP)
        r8 = (xs - x8.astype(np.float32)).astype(F8NP)

        def pack_nm(a):  # node-major [SL, NN, DIM] -> [SL, 128, NT*DIM]
            return np.ascontiguousarray(
                a.reshape(SL, NT, 128, DIM).transpose(0, 2, 1, 3).reshape(SL, 128, -1)
            )

        def pack_fm(a):  # feature-major [SL, NN, DIM] -> [SL, 128, EC*NN]
            at = a.transpose(0, 2, 1)                          # [SL, DIM, NN]
            return at.reshape(SL, EC, 128, NN).transpose(0, 2, 1, 3).reshape(
                SL, 128, -1
            )

        xtr = np.ascontiguousarray(
            np.stack([pack_fm(x8), pack_fm(r8)], axis=2)
        )                                                      # [SL, 128, 2, EC*NN]
        m = {"x8": pack_nm(x8), "xtr": xtr, "mh": mh}
        if GRAM_CORR:
            m["r8"] = pack_nm(r8)
        in_maps.append(m)
    return in_maps


def run(x, Q, K, alpha, beta, **spmd_kwargs):
    """Build, run on 8 cores, gather. Returns (out, BassKernelResults, nc)."""
    in_maps = _host_prep(x, Q, K, alpha, beta)
    nc = build_nc()
    res = run_bass_kernel_spmd(nc, in_maps, core_ids=list(range(N_CORES)), **spmd_kwargs)

    xf = np.asarray(x, dtype=np.float32)
    sa = (1.0 / (1.0 + np.exp(-np.asarray(alpha, dtype=np.float32)))).reshape(HEAD)
    sa_cols = np.repeat(sa, HD).astype(np.float32)            # [256]

    # device out: [SL, 128, EC, NN] bf16 = attn^T; host restores node-major
    attn = np.empty((BT, NN, DIM), dtype=np.float32)
    for c in range(N_CORES):
        a = res.results[c]["out"].astype(np.float32)          # [SL,128,EC,NN]
        attn[c * SL : (c + 1) * SL] = (
            a.transpose(0, 2, 1, 3).reshape(SL, DIM, NN).transpose(0, 2, 1)
        )
    out = sa_cols[None, None, None, :] * xf.reshape(B, T, NN, DIM) + attn.reshape(
        B, T, NN, DIM
    )
    return np.ascontiguousarray(out), res, nc


def kernel(x, Q, K, alpha, beta):
    out, _, _ = run(x, Q, K, alpha, beta)
    return out
